# revision 38
# baseline (speedup 1.0000x reference)
"""Trainium2 Bass kernel for nn_AttentionGCN (TGCN: GRU over GCN message passing).

Strategy (8 NeuronCores, graph/data parallel by destination node):
  - prop(Xt @ W) == prop(Xt) @ W  (propagation commutes with feature projection),
    so ONE SpMM over the raw [N, 96] features replaces 36 SpMMs over [N, 32].
  - Nodes partitioned across 8 cores (12500 each); edges placed by destination
    core; x table (node-major fp16 [N, 96], 192 B rows) replicated per core so
    source gathers are local indirect DMAs.
  - deg/dinv computed on-device per core; dinv AllGather'd (tiny collective),
    then ytab = dinv[n]*x[n] built on device in fp16, so the per-edge gather
    already carries the dinv[src] factor.
  - SpMM: per 128-dst block, one indirect row gather (fp16, 192 B descriptors)
    + per-128-edge-tile selection matmuls accumulating [96, 128] in PSUM.
    Selection matrices are HOST-built (one-hot x edge weight -> fp16) and
    narrowed to K columns: 128 dst-sorted edges only span a few dst ranks, so
    each tile's matmul writes a [96, K] window at a host-chosen rank offset
    (a zeroing matmul opens each block's accumulation group).  The PSUM->SBUF
    copy applies dinv[dst] via a broadcast tile.
  - GRU: 2-group node packing (G2): [H(64) | ax(16)] stacked rhs so one fused
    [80,128] matmul produces Z|R for two node groups; fp16 elementwise at
    64 partitions; two node-halves (SC) so the first half's GRU overlaps the
    second half's SpMM.
"""

import numpy as np
from contextlib import ExitStack

import concourse.bass as bass
import concourse.bacc as bacc
import concourse.tile as tile
import concourse.mybir as mybir

F32 = mybir.dt.float32
F16 = mybir.dt.float16
I32 = mybir.dt.int32
ALU = mybir.AluOpType
ACTF = mybir.ActivationFunctionType

CFG = dict(
    ncores=8,
    npc=12500,        # real nodes per core
    nblk=98,          # dst blocks of 128 (=> padded 12544 nodes/core)
    f_in=8,
    p=12,
    out=32,
    gch=512,          # GRU matmul chunk (free dim; psum bank = 512 f32)
    ytr=34,           # ytab build rows per partition per chunk (782 = 23*34)
    scb=(13, 13, 13, 10),  # GRU super-chunk widths in dst blocks (per group)
    lag=3,            # wavefront lag (t-steps) between successive SCs
)


def host_prep(x, edge_index, edge_weight, cfg):
    """Shard + lay out inputs (index manipulation & dtype casts only)."""
    ncores, npc, nblk = cfg["ncores"], cfg["npc"], cfg["nblk"]
    f_in, p = cfg["f_in"], cfg["p"]
    feat = f_in * p
    npcp = nblk * 128
    n = ncores * npc
    npad = ((n + 4351) // 4352) * 4352  # 100096 = 782*128, divisible by 128*34

    x = np.asarray(x, dtype=np.float32)
    src_g = np.asarray(edge_index[0], dtype=np.int64)
    dst_g = np.asarray(edge_index[1], dtype=np.int64)
    w_g = np.asarray(edge_weight, dtype=np.float32)

    # node-major fp16 table [npad, 96], feats t-major (t*f_in + f)
    x16 = np.zeros((npad, feat), dtype=np.float16)
    x16[:n] = np.ascontiguousarray(np.transpose(x, (0, 2, 1))).reshape(n, feat)

    core_of = dst_g // npc

    per_core = []
    maxblk = 0
    maxdeg = 0
    for c in range(ncores):
        m = core_of == c
        cs = src_g[m]
        cd = dst_g[m] - c * npc
        cw = w_g[m]
        # self loops (w=1) as ordinary edges
        cs = np.concatenate([cs, np.arange(npc, dtype=np.int64) + c * npc])
        cd = np.concatenate([cd, np.arange(npc, dtype=np.int64)])
        cw = np.concatenate([cw, np.ones(npc, dtype=np.float32)])

        order_d = np.argsort(cd, kind="stable")
        ds = cd[order_d]
        ws = cw[order_d]
        start = np.searchsorted(ds, np.arange(npc))
        slot = np.arange(len(ds)) - start[ds]
        deg_cnt = np.bincount(ds, minlength=npc)
        maxdeg = max(maxdeg, int(deg_cnt.max()))

        blk = ds >> 7
        rank = ds & 127
        cnt = np.bincount(blk, minlength=nblk)
        maxblk = max(maxblk, int(cnt.max()))
        per_core.append((cs[order_d], ws, blk, rank, cnt, ds, slot))

    eb = (maxblk + 127) // 128
    dmax = maxdeg

    # --- per (block, tile) rank windows, uniform across cores -------------
    # r0[b, t] = min rank in tile (over cores); K covers the max span.
    rmin = np.full((nblk, eb), 128, dtype=np.int64)
    rmax = np.full((nblk, eb), -1, dtype=np.int64)
    core_bt = []
    for c in range(ncores):
        cs, ws, blk, rank, cnt, ds, slot = per_core[c]
        bstart = np.zeros(nblk, dtype=np.int64)
        bstart[1:] = np.cumsum(cnt)[:-1]
        j = np.arange(len(ds)) - bstart[blk]
        pp = j % 128
        tt = j // 128
        np.minimum.at(rmin, (blk, tt), rank)
        np.maximum.at(rmax, (blk, tt), rank)
        core_bt.append((pp, tt))
    r0 = np.where(rmin > rmax, 0, rmin)
    kreq = int(np.max(np.where(rmax >= 0, rmax - r0 + 1, 1)))
    K = 16
    while K < kreq:
        K *= 2
    assert K <= 128
    r0 = np.minimum(r0, 128 - K)  # clamp; spans still fit (rank <= 127)

    in_maps = []
    for c in range(ncores):
        cs, ws, blk, rank, cnt, ds, slot = per_core[c]
        pp, tt = core_bt[c]

        gsrc = np.zeros((128, nblk, eb), dtype=np.int32)
        gsrc[pp, blk, tt] = cs.astype(np.int32)
        gsrc = gsrc.reshape(128, nblk * eb)
        st16 = np.zeros((128, nblk, eb * K), dtype=np.float16)
        st16[pp, blk, tt * K + (rank - r0[blk, tt])] = ws
        st16 = st16.reshape(128, nblk * eb * K)

        wdegT = np.zeros((nblk, 128, dmax), dtype=np.float32)
        wdegT[blk, rank, slot] = ws
        if npcp > npc:
            wdegT.reshape(npcp, dmax)[npc:, 0] = 1.0  # virtual pad nodes: deg=1
        wdegT = wdegT.reshape(nblk, 128 * dmax)

        xcol = np.zeros(npcp, dtype=np.float32)
        xcol[:npc] = x[c * npc:(c + 1) * npc, 1, -1]
        xcol2 = xcol.reshape(2, npcp // 2)

        in_maps.append(dict(
            x16=x16,
            gsrc=gsrc,
            st16=st16,
            wdegT=wdegT,
            xcol2=xcol2,
        ))
    return in_maps, eb, dmax, npad, K, r0


def host_weights(params, cfg):
    """Pack the small weights into one array (layout only; folding on device).
    Column map: 0:32 Lz | 32:64 Lr | 64:96 Lh | 96:104 Wz.T | 104:112 Wr.T |
    112:120 Wh.T | 120..125 bz br bh lbz lbr lbh | 126 Wp | 127 bp(row0) |
    128:128+p att(row0)."""
    out, f_in, p = cfg["out"], cfg["f_in"], cfg["p"]
    wpack = np.zeros((2 * out, 4 * out + f_in * 3 + 8 + p), dtype=np.float32)
    wpack[:, 0:out] = params["Lz"]
    wpack[:, out:2 * out] = params["Lr"]
    wpack[:, 2 * out:3 * out] = params["Lh"]
    c = 3 * out
    wpack[0:out, c:c + f_in] = np.asarray(params["Wz"]).T
    wpack[0:out, c + f_in:c + 2 * f_in] = np.asarray(params["Wr"]).T
    wpack[0:out, c + 2 * f_in:c + 3 * f_in] = np.asarray(params["Wh"]).T
    c += 3 * f_in
    for i, k in enumerate(("bz", "br", "bh", "lbz", "lbr", "lbh")):
        wpack[0:out, c + i] = np.asarray(params[k]).reshape(out)
    wpack[0:out, c + 6] = np.asarray(params["Wp"]).reshape(out)
    wpack[0, c + 7] = float(np.asarray(params["bp"]).reshape(()))
    wpack[0, c + 8:c + 8 + p] = np.asarray(params["att"]).reshape(p)
    return {"wpack": wpack}


def build_graph(cfg, eb, dmax, npad, K, r0):
    ncores, npc, nblk = cfg["ncores"], cfg["npc"], cfg["nblk"]
    f_in, p, out = cfg["f_in"], cfg["p"], cfg["out"]
    gch, ytr = cfg["gch"], cfg["ytr"]
    feat = f_in * p
    npcp = nblk * 128
    half = npcp // 2          # 6272 nodes per G2 group
    gblk = nblk // 2          # 49 blocks per group
    scw = half // 2           # 3136 super-chunk width
    n = ncores * npc
    nyc = npad // (128 * ytr)  # ytab build chunks (23)
    nc = bacc.Bacc(monotonic_sem_count=0)

    x16 = nc.declare_dram_parameter("x16", [npad, feat], F16, isOutput=False)
    gsrc = nc.declare_dram_parameter("gsrc", [128, nblk * eb], I32, isOutput=False)
    st16 = nc.declare_dram_parameter("st16", [128, nblk * eb * K], F16, isOutput=False)
    wdegT = nc.declare_dram_parameter("wdegT", [nblk, 128 * dmax], F32, isOutput=False)
    xcol2 = nc.declare_dram_parameter("xcol2", [2, half], F32, isOutput=False)
    wcols = 4 * out + f_in * 3 + 8 + p
    wpack = nc.declare_dram_parameter("wpack", [2 * out, wcols], F32, isOutput=False)
    out_ext = nc.declare_dram_parameter("out", [2, half], F16, isOutput=True)

    dinv_shard = nc.dram_tensor("dinv_shard", [npcp], F32)
    dinv_all = nc.dram_tensor("dinv_all", [npcp * ncores], F32, addr_space="Shared")
    dinv_glob = nc.dram_tensor("dinv_glob", [npad], F32)

    with tile.TileContext(nc) as tc, ExitStack() as ctx:
        cpool = ctx.enter_context(tc.tile_pool(name="const", bufs=1))
        # ---------------- stage 0: constants + weight folding ----------------
        wpk = cpool.tile([2 * out, wcols], F32)
        nc.sync.dma_start(wpk[:], wpack[:])
        cW = 3 * out
        cB = cW + 3 * f_in
        wsb = {
            "Lz": wpk[:, 0:out], "Lr": wpk[:, out:2 * out], "Lh": wpk[:, 2 * out:3 * out],
            "WzT": wpk[0:out, cW:cW + f_in],
            "WrT": wpk[0:out, cW + f_in:cW + 2 * f_in],
            "WhT": wpk[0:out, cW + 2 * f_in:cW + 3 * f_in],
            "bz": wpk[0:out, cB:cB + 1], "br": wpk[0:out, cB + 1:cB + 2],
            "bh": wpk[0:out, cB + 2:cB + 3], "lbz": wpk[0:out, cB + 3:cB + 4],
            "lbr": wpk[0:out, cB + 4:cB + 5], "lbh": wpk[0:out, cB + 5:cB + 6],
            "Wp": wpk[0:out, cB + 6:cB + 7], "bp": wpk[0:1, cB + 7:cB + 8],
            "att": wpk[0:1, cB + 8:cB + 8 + p],
        }

        UU = cpool.tile([f_in, 2 * out], F32)   # [Uz | Ur] = Wx @ Lx[:out]
        Uh = cpool.tile([f_in, out], F32)
        VV = cpool.tile([out, 2 * out], F32)    # [Vz | Vr] = Lx[out:]
        Vh32 = cpool.tile([out, out], F32)
        cbzr = cpool.tile([2 * out, 1], F32)    # [cbz; cbr]
        cbh = cpool.tile([out, 1], F32)
        pmat64 = cpool.tile([2 * out, p], F32)
        # stacked fp16 GRU weights (G2 rows: 0:32 Hg0 | 32:64 Hg1 | 64:72 axg0
        # | 72:80 axg1; ZR cols: 0:32 Zg0 | 32:64 Zg1 | 64:96 Rg0 | 96:128 Rg1)
        UVzr2 = cpool.tile([80, 128], F16)
        U2hs = cpool.tile([80, 2 * out], F16)  # rows 64:80 hold [Uh_g0; Uh_g1]
        V2hs = cpool.tile([2 * out, 2 * out], F16)
        wp2s = cpool.tile([2 * out, 2], F16)
        biasZ2 = cpool.tile([2 * out, 1], F32)
        biasR2 = cpool.tile([2 * out, 1], F32)
        cbh2 = cpool.tile([2 * out, 1], F32)
        bp2 = cpool.tile([2, 1], F32)

        with tc.tile_pool(name="foldp", bufs=2, space="PSUM") as fpool:
            # Ux = Wx @ Lx[:out]  ->  lhsT = Wx.T, rhs = Lx[:out]
            for wt, lt, dst in ((("WzT"), "Lz", UU[:, 0:out]),
                                (("WrT"), "Lr", UU[:, out:2 * out]),
                                (("WhT"), "Lh", Uh[:, :])):
                ps = fpool.tile([f_in, out], F32, tag="pu")
                nc.tensor.matmul(ps[:], lhsT=wsb[wt][:], rhs=wsb[lt][0:out, :],
                                 start=True, stop=True)
                nc.vector.tensor_copy(dst, ps[:])
            nc.vector.tensor_copy(VV[:, 0:out], wsb["Lz"][out:2 * out, :])
            nc.vector.tensor_copy(VV[:, out:2 * out], wsb["Lr"][out:2 * out, :])
            nc.vector.tensor_copy(Vh32[:, :], wsb["Lh"][out:2 * out, :])
            # cbx = Lx[:out].T @ bx + lbx   [out, 1]
            for lt, bt, lbt, dst in (("Lz", "bz", "lbz", cbzr[0:out, :]),
                                     ("Lr", "br", "lbr", cbzr[out:2 * out, :]),
                                     ("Lh", "bh", "lbh", cbh[:, :])):
                ps = fpool.tile([out, 1], F32, tag="pb")
                nc.tensor.matmul(ps[:], lhsT=wsb[lt][0:out, :], rhs=wsb[bt][:],
                                 start=True, stop=True)
                tmpb = cpool.tile([out, 1], F32, tag="tmpb", name="tmpb")
                nc.vector.tensor_add(tmpb[:], ps[:], wsb[lbt][:])
                nc.vector.tensor_copy(dst, tmpb[:])
            # probs = softmax(att) -> pmat64 [64, p]
            amax = cpool.tile([1, 1], F32)
            nc.vector.tensor_reduce(amax[:], wsb["att"][:], axis=mybir.AxisListType.X,
                                    op=ALU.max)
            namax = cpool.tile([1, 1], F32)
            nc.vector.tensor_scalar(out=namax[:], in0=amax[:], scalar1=-1.0,
                                    scalar2=None, op0=ALU.mult)
            aexp = cpool.tile([1, p], F32)
            nc.scalar.activation(aexp[:], wsb["att"][:], ACTF.Exp, bias=namax[0:1, 0:1])
            asum = cpool.tile([1, 1], F32)
            nc.vector.tensor_reduce(asum[:], aexp[:], axis=mybir.AxisListType.X,
                                    op=ALU.add)
            arcp = cpool.tile([1, 1], F32)
            nc.vector.reciprocal(arcp[:], asum[:])
            probs = cpool.tile([1, p], F32)
            nc.vector.tensor_scalar(out=probs[:], in0=aexp[:], scalar1=arcp[0:1, 0:1],
                                    scalar2=None, op0=ALU.mult)
            ones64 = cpool.tile([1, 2 * out], F32)
            nc.gpsimd.memset(ones64[:], 1.0)
            psp = fpool.tile([2 * out, p], F32, tag="pp")
            nc.tensor.matmul(psp[:], lhsT=ones64[:], rhs=probs[:], start=True, stop=True)
            nc.vector.tensor_copy(pmat64[:], psp[:])

        # stacked fp16 weights (copies convert f32 -> fp16)
        nc.vector.memset(UVzr2[:], 0.0)
        nc.vector.memset(U2hs[:], 0.0)
        nc.vector.memset(V2hs[:], 0.0)
        nc.vector.memset(wp2s[:], 0.0)
        Uz, Ur = UU[:, 0:out], UU[:, out:2 * out]
        Vz, Vr = VV[:, 0:out], VV[:, out:2 * out]
        nc.vector.tensor_copy(UVzr2[0:32, 0:32], Vz)
        nc.vector.tensor_copy(UVzr2[0:32, 64:96], Vr)
        nc.vector.tensor_copy(UVzr2[32:64, 32:64], Vz)
        nc.vector.tensor_copy(UVzr2[32:64, 96:128], Vr)
        # 8-row pieces: engine ops need 32-aligned partition bases, so stage
        # fp16 copies at base 0 and DMA them into the odd-base rows.
        Uz16 = cpool.tile([f_in, out], F16)
        Ur16 = cpool.tile([f_in, out], F16)
        Uh16 = cpool.tile([f_in, out], F16)
        nc.vector.tensor_copy(Uz16[:], Uz)
        nc.vector.tensor_copy(Ur16[:], Ur)
        nc.vector.tensor_copy(Uh16[:], Uh[:])
        nc.sync.dma_start(UVzr2[64:72, 0:32], Uz16[:])
        nc.sync.dma_start(UVzr2[64:72, 64:96], Ur16[:])
        nc.sync.dma_start(UVzr2[72:80, 32:64], Uz16[:])
        nc.sync.dma_start(UVzr2[72:80, 96:128], Ur16[:])
        nc.sync.dma_start(U2hs[64:72, 0:32], Uh16[:])
        nc.sync.dma_start(U2hs[72:80, 32:64], Uh16[:])
        nc.vector.tensor_copy(V2hs[0:32, 0:32], Vh32[:])
        nc.vector.tensor_copy(V2hs[32:64, 32:64], Vh32[:])
        nc.vector.tensor_copy(wp2s[0:32, 0:1], wsb["Wp"])
        nc.vector.tensor_copy(wp2s[32:64, 1:2], wsb["Wp"])
        nc.vector.tensor_copy(biasZ2[0:32, :], cbzr[0:32, :])
        nc.vector.tensor_copy(biasZ2[32:64, :], cbzr[0:32, :])
        nc.vector.tensor_copy(biasR2[0:32, :], cbzr[32:64, :])
        nc.vector.tensor_copy(biasR2[32:64, :], cbzr[32:64, :])
        nc.vector.tensor_copy(cbh2[0:32, :], cbh[:])
        nc.vector.tensor_copy(cbh2[32:64, :], cbh[:])
        nc.vector.tensor_copy(bp2[0:1, :], wsb["bp"])
        nc.sync.dma_start(bp2[1:2, :], wsb["bp"])
        # zero lhsT/rhs for the psum-clearing matmul of each block
        z96 = cpool.tile([1, feat], F16)
        z128 = cpool.tile([1, 128], F16)
        nc.vector.memset(z96[:], 0.0)
        nc.vector.memset(z128[:], 0.0)

        # ---------------- stage 1: deg -> dinvT [nblk, 128] ----------------
        dinvT = cpool.tile([nblk, 128], F32)
        with tc.tile_pool(name="degp", bufs=1) as dpool:
            wdg = dpool.tile([nblk, 128 * dmax], F32)
            nc.scalar.dma_start(wdg[:], wdegT[:])
            deg = dpool.tile([nblk, 128], F32)
            nc.vector.tensor_reduce(
                deg[:, :, None],
                wdg[:].rearrange("b (q d) -> b q d", d=dmax),
                axis=mybir.AxisListType.X, op=ALU.add)
            sq = dpool.tile([nblk, 128], F32)
            nc.scalar.activation(sq[:], deg[:], ACTF.Sqrt)
            nc.vector.reciprocal(dinvT[:], sq[:])
            # node-major pack: dinv_shard[b*128 + q] = dinvT[b, q]  (contiguous
            # per partition). NOTE: must be a GPSIMD (SWDGE) DMA — sync/HWDGE
            # DMAs writing a collective's input buffer deadlock in NRT.
            nc.gpsimd.dma_start(dinv_shard[:].rearrange("(b q) -> b q", q=128),
                                dinvT[:])

        # ---------------- stage 2: allgather dinv ---------------------------
        nc.gpsimd.collective_compute(
            "AllGather", ALU.bypass,
            ins=[dinv_shard[:]], outs=[dinv_all[:]],
            replica_groups=[list(range(ncores))])
        for c in range(ncores):
            nc.sync.dma_start(out=dinv_glob[c * npc:(c + 1) * npc],
                              in_=dinv_all[c * npcp:c * npcp + npc])
        if npad > n:  # zero the padding tail
            zt = cpool.tile([1, npad - n], F32)
            nc.vector.memset(zt[:], 0.0)
            nc.sync.dma_start(out=dinv_glob[n:npad, None], in_=zt[0:1, :])

        # ---------------- stage 2c: dinvd_all [96, npcp] fp16 ---------------
        # (local-only; overlaps the allgather) dinvd_all[f, n] = dinv[dst n]
        dinvd_all = cpool.tile([feat, npcp], F16)
        ones96f = cpool.tile([1, feat], F32)
        nc.gpsimd.memset(ones96f[:], 1.0)
        with tc.tile_pool(name="psdv", bufs=4, space="PSUM") as pdv, \
             tc.tile_pool(name="drow", bufs=2) as drp:
            for ci in range(npcp // 448):
                csl = slice(ci * 448, (ci + 1) * 448)
                drc = drp.tile([1, 448], F32, tag="drc")
                nc.sync.dma_start(drc[:], dinv_shard[None, csl])
                psd = pdv.tile([feat, 448], F32, tag="psd")
                nc.tensor.matmul(psd[:], lhsT=ones96f[:], rhs=drc[:],
                                 start=True, stop=True)
                nc.vector.tensor_copy(dinvd_all[:, csl], psd[:])

        # ---------------- stage 3: SpMM (gather + selection matmuls) --------
        # axt[t*8+f, node] = (A @ Y)[node, t*8+f] * dinv[dst]
        axt = cpool.tile([feat, npcp], F16)

        scb = cfg["scb"]
        lag = cfg["lag"]
        nsc = len(scb)
        assert sum(scb) == gblk
        scb0 = [sum(scb[:i]) for i in range(nsc + 1)]  # block offsets per group

        # block order: quarter q = g0-span + g1-span (GRU SC q depends on it)
        quarters = []
        for q in range(nsc):
            quarters.append(list(range(scb0[q], scb0[q + 1])) +
                            list(range(gblk + scb0[q], gblk + scb0[q + 1])))
        ord_blocks = [b for qs in quarters for b in qs]
        assert sorted(ord_blocks) == list(range(nblk))

        gpool = ctx.enter_context(tc.tile_pool(name="gat", bufs=3))
        bpool = ctx.enter_context(tc.tile_pool(name="bld", bufs=2))
        pgpool = ctx.enter_context(tc.tile_pool(name="ps_g", bufs=2, space="PSUM"))

        def emit_block(idxb, stb, j, b):
            Y = gpool.tile([128, eb * feat], F16, tag="Y")
            nc.gpsimd.indirect_dma_start(
                out=Y[:], out_offset=None,
                in_=x16[:, :],
                in_offset=bass.IndirectOffsetOnAxis(ap=idxb[:, j, :], axis=0))
            Yr = Y[:].rearrange("q (e f) -> q e f", f=feat)
            # per-edge dinv[src] (4 B gather) folded into the st tile
            dsr = gpool.tile([128, eb], F32, tag="dsr")
            nc.gpsimd.indirect_dma_start(
                out=dsr[:], out_offset=None,
                in_=dinv_glob[:, None],
                in_offset=bass.IndirectOffsetOnAxis(ap=idxb[:, j, :], axis=0))
            stj = stb[:, j, :].rearrange("q (e k) -> q e k", k=K)
            nc.vector.tensor_tensor(
                out=stj, in0=stj,
                in1=dsr[:, :, None].to_broadcast([128, eb, K]), op=ALU.mult)
            ps = pgpool.tile([feat, 128], F32, tag="psA")
            nc.tensor.matmul(ps[:], lhsT=z96[:], rhs=z128[:],
                             start=True, stop=False)
            for t in range(eb):
                w0 = int(r0[b, t])
                nc.tensor.matmul(ps[:, w0:w0 + K], lhsT=Yr[:, t, :],
                                 rhs=stb[:, j, t * K:(t + 1) * K],
                                 start=False, stop=(t == eb - 1))
            nc.vector.tensor_tensor(
                out=axt[:, b * 128:(b + 1) * 128], in0=ps[:],
                in1=dinvd_all[:, b * 128:(b + 1) * 128], op=ALU.mult)

        # one batched idx/st load per contiguous span (12-13 blocks)
        spmm_work = {}  # block -> (idxb_r, stb_r, j)
        for q in range(nsc):
            for g in range(2):
                b0 = g * gblk + scb0[q]
                nb = scb[q]
                nbmax = max(scb)
                ldq = nc.scalar if g == 0 else nc.sync
                idxb = bpool.tile([128, nbmax * eb], I32, tag="idxb")
                ldq.dma_start(idxb[:, 0:nb * eb],
                              gsrc[:, b0 * eb:(b0 + nb) * eb])
                stb = bpool.tile([128, nbmax * eb * K], F16, tag="stb")
                ldq.dma_start(
                    stb[:, 0:nb * eb * K],
                    st16[:, b0 * eb * K:(b0 + nb) * eb * K])
                idxb_r = idxb[:].rearrange("q (b e) -> q b e", e=eb)
                stb_r = stb[:].rearrange("q (b e) -> q b e", e=eb * K)
                for j in range(nb):
                    spmm_work[b0 + j] = (idxb_r, stb_r, j)

        # ---------------- stage 4: GRU over time (wavefront of SCs) ---------
        grup = ctx.enter_context(tc.tile_pool(name="gru", bufs=1))
        pzrp = ctx.enter_context(tc.tile_pool(name="ps_zr", bufs=2, space="PSUM"))
        phpool = ctx.enter_context(tc.tile_pool(name="ps_h", bufs=2, space="PSUM"))

        XH2 = grup.tile([80, half], F16)    # rows 0:64 H (g0|g1), 64:80 ax
        XZ2 = grup.tile([2 * out, half], F16)
        RH2 = grup.tile([2 * out, half], F16)
        HT2 = grup.tile([2 * out, half], F16)
        acc2 = grup.tile([2 * out, half], F16)
        nc.vector.memset(XH2[:], 0.0)
        nc.vector.memset(acc2[:], 0.0)

        def sc_chunks(sc):
            w0 = scb0[sc] * 128
            w1 = scb0[sc + 1] * 128
            ch = []
            c = w0
            while c < w1:
                cw = min(gch, w1 - c)
                ch.append((c, cw))
                c += cw
            return w0, w1, ch

        def gru_step(sc, t):
            w0, w1, chunks = sc_chunks(sc)
            scs = slice(w0, w1)
            # ax rows: axg0 -> 64:72, axg1 -> 72:80
            eng = nc.sync
            eng.dma_start(XH2[64:72, scs],
                          axt[t * f_in:(t + 1) * f_in, w0:w1])
            eng.dma_start(XH2[72:80, scs],
                          axt[t * f_in:(t + 1) * f_in, half + w0:half + w1])
            for c0, cw in chunks:
                csl = slice(c0, c0 + cw)
                pzr = pzrp.tile([128, gch], F32, tag="pzr")
                nc.tensor.matmul(pzr[:, 0:cw], lhsT=UVzr2[:], rhs=XH2[:, csl],
                                 start=True, stop=True)
                nc.scalar.activation(XZ2[:, csl], pzr[0:64, 0:cw], ACTF.Sigmoid,
                                     bias=biasZ2[:, 0:1])
                nc.scalar.activation(RH2[:, csl], pzr[64:128, 0:cw], ACTF.Sigmoid,
                                     bias=biasR2[:, 0:1])
            # RH = R * H
            nc.vector.tensor_tensor(out=RH2[:, scs], in0=RH2[:, scs],
                                    in1=XH2[0:64, scs], op=ALU.mult)
            for c0, cw in chunks:
                csl = slice(c0, c0 + cw)
                ph = phpool.tile([2 * out, gch], F32, tag="ph")
                nc.tensor.matmul(ph[:, 0:cw], lhsT=U2hs[64:80, :],
                                 rhs=XH2[64:80, csl], start=True, stop=False)
                nc.tensor.matmul(ph[:, 0:cw], lhsT=V2hs[:], rhs=RH2[:, csl],
                                 start=False, stop=True)
                nc.scalar.activation(HT2[:, csl], ph[:, 0:cw], ACTF.Tanh,
                                     bias=cbh2[:, 0:1])
            # H' = Ht + Z*(H - Ht); acc += p_t * H'   (RH2 as scratch)
            nc.vector.tensor_tensor(out=RH2[:, scs], in0=XH2[0:64, scs],
                                    in1=HT2[:, scs], op=ALU.subtract)
            nc.vector.tensor_tensor(out=RH2[:, scs], in0=XZ2[:, scs],
                                    in1=RH2[:, scs], op=ALU.mult)
            nc.vector.tensor_tensor(out=XH2[0:64, scs], in0=HT2[:, scs],
                                    in1=RH2[:, scs], op=ALU.add)
            nc.vector.scalar_tensor_tensor(
                out=acc2[:, scs], in0=XH2[0:64, scs],
                scalar=pmat64[:, t:t + 1], in1=acc2[:, scs],
                op0=ALU.mult, op1=ALU.add)

        def gru_head(sc):
            w0, w1, chunks = sc_chunks(sc)
            scs = slice(w0, w1)
            nc.scalar.activation(HT2[:, scs], acc2[:, scs], ACTF.Relu)
            with tc.tile_pool(name=f"ps_d{sc}", bufs=1, space="PSUM") as pdpool, \
                 tc.tile_pool(name=f"ov{sc}", bufs=3) as ovpool:
                for c0, cw in chunks:
                    csl = slice(c0, c0 + cw)
                    pd = pdpool.tile([2, gch], F32, tag="pd")
                    nc.tensor.matmul(pd[:, 0:cw], lhsT=wp2s[:], rhs=HT2[:, csl],
                                     start=True, stop=True)
                    xcc = ovpool.tile([2, gch], F32, tag="xcc")
                    nc.sync.dma_start(xcc[:, 0:cw], xcol2[:, csl])
                    o2c = ovpool.tile([2, gch], F16, tag="o2c")
                    nc.vector.tensor_tensor(out=o2c[:, 0:cw], in0=pd[:, 0:cw],
                                            in1=xcc[:, 0:cw], op=ALU.add)
                    nc.vector.tensor_scalar(out=o2c[:, 0:cw], in0=o2c[:, 0:cw],
                                            scalar1=bp2[:, 0:1], scalar2=0.0,
                                            op0=ALU.add, op1=ALU.max)
                    nc.sync.dma_start(out_ext[:, csl], o2c[:, 0:cw])

        # wavefront: SC sc runs t-steps at slots sc*lag + t; quarter q+1's
        # SpMM blocks are emitted at the start of slot q*lag.
        emitted_q = 0
        for b in quarters[0]:
            emit_block(*spmm_work[b], b)
        emitted_q = 1
        n_slots = (nsc - 1) * lag + p
        for k in range(n_slots):
            if k % lag == 0 and emitted_q < nsc:
                for b in quarters[emitted_q]:
                    emit_block(*spmm_work[b], b)
                emitted_q += 1
            for sc in range(nsc):
                t = k - sc * lag
                if 0 <= t < p:
                    gru_step(sc, t)
                if t == p - 1:
                    gru_head(sc)

    return nc


TRACE = False
LAST_EXEC_TIME_NS = None
LAST_RESULT = None


def kernel(**inputs):
    global LAST_EXEC_TIME_NS, LAST_RESULT
    cfg = CFG
    x = np.asarray(inputs["x"], dtype=np.float32)
    in_maps, eb, dmax, npad, K, r0 = host_prep(x, inputs["edge_index"],
                                               inputs["edge_weight"], cfg)
    w = host_weights(inputs, cfg)
    for m in in_maps:
        m.update(w)
    nc = build_graph(cfg, eb, dmax, npad, K, r0)
    nc.finalize()

    from concourse.bass_utils import run_bass_kernel_spmd
    res = run_bass_kernel_spmd(nc, in_maps, core_ids=list(range(cfg["ncores"])),
                               trace=TRACE)
    LAST_EXEC_TIME_NS = res.exec_time_ns
    LAST_RESULT = res
    npc = cfg["npc"]
    outs = []
    for c in range(cfg["ncores"]):
        o = np.asarray(res.results[c]["out"], dtype=np.float32)  # [2, half]
        outs.append(o.reshape(-1)[:npc])
    return np.concatenate(outs).reshape(-1, 1).astype(np.float32)


# revision 39
# speedup vs baseline: 1.0008x; 1.0008x over previous
"""Trainium2 Bass kernel for nn_AttentionGCN (TGCN: GRU over GCN message passing).

Strategy (8 NeuronCores, graph/data parallel by destination node):
  - prop(Xt @ W) == prop(Xt) @ W  (propagation commutes with feature projection),
    so ONE SpMM over the raw [N, 96] features replaces 36 SpMMs over [N, 32].
  - Nodes partitioned across 8 cores (12500 each); edges placed by destination
    core; x table (node-major fp16 [N, 96], 192 B rows) replicated per core so
    source gathers are local indirect DMAs.
  - deg/dinv computed on-device per core; dinv AllGather'd (tiny collective),
    then ytab = dinv[n]*x[n] built on device in fp16, so the per-edge gather
    already carries the dinv[src] factor.
  - SpMM: per 128-dst block, one indirect row gather (fp16, 192 B descriptors)
    + per-128-edge-tile selection matmuls accumulating [96, 128] in PSUM.
    Selection matrices are HOST-built (one-hot x edge weight -> fp16) and
    narrowed to K columns: 128 dst-sorted edges only span a few dst ranks, so
    each tile's matmul writes a [96, K] window at a host-chosen rank offset
    (a zeroing matmul opens each block's accumulation group).  The PSUM->SBUF
    copy applies dinv[dst] via a broadcast tile.
  - GRU: 2-group node packing (G2): [H(64) | ax(16)] stacked rhs so one fused
    [80,128] matmul produces Z|R for two node groups; fp16 elementwise at
    64 partitions; two node-halves (SC) so the first half's GRU overlaps the
    second half's SpMM.
"""

import numpy as np
from contextlib import ExitStack

import concourse.bass as bass
import concourse.bacc as bacc
import concourse.tile as tile
import concourse.mybir as mybir

F32 = mybir.dt.float32
F16 = mybir.dt.float16
I32 = mybir.dt.int32
ALU = mybir.AluOpType
ACTF = mybir.ActivationFunctionType

CFG = dict(
    ncores=8,
    npc=12500,        # real nodes per core
    nblk=98,          # dst blocks of 128 (=> padded 12544 nodes/core)
    f_in=8,
    p=12,
    out=32,
    gch=512,          # GRU matmul chunk (free dim; psum bank = 512 f32)
    ytr=34,           # ytab build rows per partition per chunk (782 = 23*34)
    scb=(13, 13, 13, 10),  # GRU super-chunk widths in dst blocks (per group)
    lag=3,            # wavefront lag (t-steps) between successive SCs
)


def host_prep(x, edge_index, edge_weight, cfg):
    """Shard + lay out inputs (index manipulation & dtype casts only)."""
    ncores, npc, nblk = cfg["ncores"], cfg["npc"], cfg["nblk"]
    f_in, p = cfg["f_in"], cfg["p"]
    feat = f_in * p
    npcp = nblk * 128
    n = ncores * npc
    npad = ((n + 4351) // 4352) * 4352  # 100096 = 782*128, divisible by 128*34

    x = np.asarray(x, dtype=np.float32)
    src_g = np.asarray(edge_index[0], dtype=np.int64)
    dst_g = np.asarray(edge_index[1], dtype=np.int64)
    w_g = np.asarray(edge_weight, dtype=np.float32)

    # node-major fp16 table [npad, 96], feats t-major (t*f_in + f)
    x16 = np.zeros((npad, feat), dtype=np.float16)
    x16[:n] = np.ascontiguousarray(np.transpose(x, (0, 2, 1))).reshape(n, feat)

    core_of = dst_g // npc

    per_core = []
    maxblk = 0
    maxdeg = 0
    for c in range(ncores):
        m = core_of == c
        cs = src_g[m]
        cd = dst_g[m] - c * npc
        cw = w_g[m]
        # self loops (w=1) as ordinary edges
        cs = np.concatenate([cs, np.arange(npc, dtype=np.int64) + c * npc])
        cd = np.concatenate([cd, np.arange(npc, dtype=np.int64)])
        cw = np.concatenate([cw, np.ones(npc, dtype=np.float32)])

        order_d = np.argsort(cd, kind="stable")
        ds = cd[order_d]
        ws = cw[order_d]
        start = np.searchsorted(ds, np.arange(npc))
        slot = np.arange(len(ds)) - start[ds]
        deg_cnt = np.bincount(ds, minlength=npc)
        maxdeg = max(maxdeg, int(deg_cnt.max()))

        blk = ds >> 7
        rank = ds & 127
        cnt = np.bincount(blk, minlength=nblk)
        maxblk = max(maxblk, int(cnt.max()))
        per_core.append((cs[order_d], ws, blk, rank, cnt, ds, slot))

    eb = (maxblk + 127) // 128
    dmax = maxdeg

    # --- per (block, tile) rank windows, uniform across cores -------------
    # r0[b, t] = min rank in tile (over cores); K covers the max span.
    rmin = np.full((nblk, eb), 128, dtype=np.int64)
    rmax = np.full((nblk, eb), -1, dtype=np.int64)
    core_bt = []
    for c in range(ncores):
        cs, ws, blk, rank, cnt, ds, slot = per_core[c]
        bstart = np.zeros(nblk, dtype=np.int64)
        bstart[1:] = np.cumsum(cnt)[:-1]
        j = np.arange(len(ds)) - bstart[blk]
        pp = j % 128
        tt = j // 128
        np.minimum.at(rmin, (blk, tt), rank)
        np.maximum.at(rmax, (blk, tt), rank)
        core_bt.append((pp, tt))
    r0 = np.where(rmin > rmax, 0, rmin)
    kreq = int(np.max(np.where(rmax >= 0, rmax - r0 + 1, 1)))
    K = 16
    while K < kreq:
        K *= 2
    assert K <= 128
    r0 = np.minimum(r0, 128 - K)  # clamp; spans still fit (rank <= 127)

    in_maps = []
    for c in range(ncores):
        cs, ws, blk, rank, cnt, ds, slot = per_core[c]
        pp, tt = core_bt[c]

        gsrc = np.zeros((128, nblk, eb), dtype=np.int32)
        gsrc[pp, blk, tt] = cs.astype(np.int32)
        gsrc = gsrc.reshape(128, nblk * eb)
        st16 = np.zeros((128, nblk, eb * K), dtype=np.float16)
        st16[pp, blk, tt * K + (rank - r0[blk, tt])] = ws
        st16 = st16.reshape(128, nblk * eb * K)

        wdegT = np.zeros((nblk, 128, dmax), dtype=np.float32)
        wdegT[blk, rank, slot] = ws
        if npcp > npc:
            wdegT.reshape(npcp, dmax)[npc:, 0] = 1.0  # virtual pad nodes: deg=1
        wdegT = wdegT.reshape(nblk, 128 * dmax)

        xcol = np.zeros(npcp, dtype=np.float32)
        xcol[:npc] = x[c * npc:(c + 1) * npc, 1, -1]
        xcol2 = xcol.reshape(2, npcp // 2)

        in_maps.append(dict(
            x16=x16,
            gsrc=gsrc,
            st16=st16,
            wdegT=wdegT,
            xcol2=xcol2,
        ))
    return in_maps, eb, dmax, npad, K, r0


def host_weights(params, cfg):
    """Pack the small weights into one array (layout only; folding on device).
    Column map: 0:32 Lz | 32:64 Lr | 64:96 Lh | 96:104 Wz.T | 104:112 Wr.T |
    112:120 Wh.T | 120..125 bz br bh lbz lbr lbh | 126 Wp | 127 bp(row0) |
    128:128+p att(row0)."""
    out, f_in, p = cfg["out"], cfg["f_in"], cfg["p"]
    wpack = np.zeros((2 * out, 4 * out + f_in * 3 + 8 + p), dtype=np.float32)
    wpack[:, 0:out] = params["Lz"]
    wpack[:, out:2 * out] = params["Lr"]
    wpack[:, 2 * out:3 * out] = params["Lh"]
    c = 3 * out
    wpack[0:out, c:c + f_in] = np.asarray(params["Wz"]).T
    wpack[0:out, c + f_in:c + 2 * f_in] = np.asarray(params["Wr"]).T
    wpack[0:out, c + 2 * f_in:c + 3 * f_in] = np.asarray(params["Wh"]).T
    c += 3 * f_in
    for i, k in enumerate(("bz", "br", "bh", "lbz", "lbr", "lbh")):
        wpack[0:out, c + i] = np.asarray(params[k]).reshape(out)
    wpack[0:out, c + 6] = np.asarray(params["Wp"]).reshape(out)
    wpack[0, c + 7] = float(np.asarray(params["bp"]).reshape(()))
    wpack[0, c + 8:c + 8 + p] = np.asarray(params["att"]).reshape(p)
    return {"wpack": wpack}


def build_graph(cfg, eb, dmax, npad, K, r0):
    ncores, npc, nblk = cfg["ncores"], cfg["npc"], cfg["nblk"]
    f_in, p, out = cfg["f_in"], cfg["p"], cfg["out"]
    gch, ytr = cfg["gch"], cfg["ytr"]
    feat = f_in * p
    npcp = nblk * 128
    half = npcp // 2          # 6272 nodes per G2 group
    gblk = nblk // 2          # 49 blocks per group
    scw = half // 2           # 3136 super-chunk width
    n = ncores * npc
    nyc = npad // (128 * ytr)  # ytab build chunks (23)
    nc = bacc.Bacc(monotonic_sem_count=0)

    x16 = nc.declare_dram_parameter("x16", [npad, feat], F16, isOutput=False)
    gsrc = nc.declare_dram_parameter("gsrc", [128, nblk * eb], I32, isOutput=False)
    st16 = nc.declare_dram_parameter("st16", [128, nblk * eb * K], F16, isOutput=False)
    wdegT = nc.declare_dram_parameter("wdegT", [nblk, 128 * dmax], F32, isOutput=False)
    xcol2 = nc.declare_dram_parameter("xcol2", [2, half], F32, isOutput=False)
    wcols = 4 * out + f_in * 3 + 8 + p
    wpack = nc.declare_dram_parameter("wpack", [2 * out, wcols], F32, isOutput=False)
    out_ext = nc.declare_dram_parameter("out", [2, half], F16, isOutput=True)

    dinv_shard = nc.dram_tensor("dinv_shard", [npcp], F32)
    dinv_all = nc.dram_tensor("dinv_all", [npcp * ncores], F32, addr_space="Shared")
    dinv_glob = nc.dram_tensor("dinv_glob", [npad], F32)

    with tile.TileContext(nc) as tc, ExitStack() as ctx:
        cpool = ctx.enter_context(tc.tile_pool(name="const", bufs=1))
        # ---------------- stage 0: constants + weight folding ----------------
        wpk = cpool.tile([2 * out, wcols], F32)
        nc.sync.dma_start(wpk[:], wpack[:])
        cW = 3 * out
        cB = cW + 3 * f_in
        wsb = {
            "Lz": wpk[:, 0:out], "Lr": wpk[:, out:2 * out], "Lh": wpk[:, 2 * out:3 * out],
            "WzT": wpk[0:out, cW:cW + f_in],
            "WrT": wpk[0:out, cW + f_in:cW + 2 * f_in],
            "WhT": wpk[0:out, cW + 2 * f_in:cW + 3 * f_in],
            "bz": wpk[0:out, cB:cB + 1], "br": wpk[0:out, cB + 1:cB + 2],
            "bh": wpk[0:out, cB + 2:cB + 3], "lbz": wpk[0:out, cB + 3:cB + 4],
            "lbr": wpk[0:out, cB + 4:cB + 5], "lbh": wpk[0:out, cB + 5:cB + 6],
            "Wp": wpk[0:out, cB + 6:cB + 7], "bp": wpk[0:1, cB + 7:cB + 8],
            "att": wpk[0:1, cB + 8:cB + 8 + p],
        }

        UU = cpool.tile([f_in, 2 * out], F32)   # [Uz | Ur] = Wx @ Lx[:out]
        Uh = cpool.tile([f_in, out], F32)
        VV = cpool.tile([out, 2 * out], F32)    # [Vz | Vr] = Lx[out:]
        Vh32 = cpool.tile([out, out], F32)
        cbzr = cpool.tile([2 * out, 1], F32)    # [cbz; cbr]
        cbh = cpool.tile([out, 1], F32)
        pmat64 = cpool.tile([2 * out, p], F32)
        # stacked fp16 GRU weights (G2 rows: 0:32 Hg0 | 32:64 Hg1 | 64:72 axg0
        # | 72:80 axg1; ZR cols: 0:32 Zg0 | 32:64 Zg1 | 64:96 Rg0 | 96:128 Rg1)
        UVzr2 = cpool.tile([80, 128], F16)
        U2hs = cpool.tile([80, 2 * out], F16)  # rows 64:80 hold [Uh_g0; Uh_g1]
        V2hs = cpool.tile([2 * out, 2 * out], F16)
        wp2s = cpool.tile([2 * out, 2], F16)
        biasZ2 = cpool.tile([2 * out, 1], F32)
        biasR2 = cpool.tile([2 * out, 1], F32)
        cbh2 = cpool.tile([2 * out, 1], F32)
        bp2 = cpool.tile([2, 1], F32)

        with tc.tile_pool(name="foldp", bufs=2, space="PSUM") as fpool:
            # Ux = Wx @ Lx[:out]  ->  lhsT = Wx.T, rhs = Lx[:out]
            for wt, lt, dst in ((("WzT"), "Lz", UU[:, 0:out]),
                                (("WrT"), "Lr", UU[:, out:2 * out]),
                                (("WhT"), "Lh", Uh[:, :])):
                ps = fpool.tile([f_in, out], F32, tag="pu")
                nc.tensor.matmul(ps[:], lhsT=wsb[wt][:], rhs=wsb[lt][0:out, :],
                                 start=True, stop=True)
                nc.vector.tensor_copy(dst, ps[:])
            nc.vector.tensor_copy(VV[:, 0:out], wsb["Lz"][out:2 * out, :])
            nc.vector.tensor_copy(VV[:, out:2 * out], wsb["Lr"][out:2 * out, :])
            nc.vector.tensor_copy(Vh32[:, :], wsb["Lh"][out:2 * out, :])
            # cbx = Lx[:out].T @ bx + lbx   [out, 1]
            for lt, bt, lbt, dst in (("Lz", "bz", "lbz", cbzr[0:out, :]),
                                     ("Lr", "br", "lbr", cbzr[out:2 * out, :]),
                                     ("Lh", "bh", "lbh", cbh[:, :])):
                ps = fpool.tile([out, 1], F32, tag="pb")
                nc.tensor.matmul(ps[:], lhsT=wsb[lt][0:out, :], rhs=wsb[bt][:],
                                 start=True, stop=True)
                tmpb = cpool.tile([out, 1], F32, tag="tmpb", name="tmpb")
                nc.vector.tensor_add(tmpb[:], ps[:], wsb[lbt][:])
                nc.vector.tensor_copy(dst, tmpb[:])
            # probs = softmax(att) -> pmat64 [64, p]
            amax = cpool.tile([1, 1], F32)
            nc.vector.tensor_reduce(amax[:], wsb["att"][:], axis=mybir.AxisListType.X,
                                    op=ALU.max)
            namax = cpool.tile([1, 1], F32)
            nc.vector.tensor_scalar(out=namax[:], in0=amax[:], scalar1=-1.0,
                                    scalar2=None, op0=ALU.mult)
            aexp = cpool.tile([1, p], F32)
            nc.scalar.activation(aexp[:], wsb["att"][:], ACTF.Exp, bias=namax[0:1, 0:1])
            asum = cpool.tile([1, 1], F32)
            nc.vector.tensor_reduce(asum[:], aexp[:], axis=mybir.AxisListType.X,
                                    op=ALU.add)
            arcp = cpool.tile([1, 1], F32)
            nc.vector.reciprocal(arcp[:], asum[:])
            probs = cpool.tile([1, p], F32)
            nc.vector.tensor_scalar(out=probs[:], in0=aexp[:], scalar1=arcp[0:1, 0:1],
                                    scalar2=None, op0=ALU.mult)
            ones64 = cpool.tile([1, 2 * out], F32)
            nc.gpsimd.memset(ones64[:], 1.0)
            psp = fpool.tile([2 * out, p], F32, tag="pp")
            nc.tensor.matmul(psp[:], lhsT=ones64[:], rhs=probs[:], start=True, stop=True)
            nc.vector.tensor_copy(pmat64[:], psp[:])

        # stacked fp16 weights (copies convert f32 -> fp16)
        nc.vector.memset(UVzr2[:], 0.0)
        nc.vector.memset(U2hs[:], 0.0)
        nc.vector.memset(V2hs[:], 0.0)
        nc.vector.memset(wp2s[:], 0.0)
        Uz, Ur = UU[:, 0:out], UU[:, out:2 * out]
        Vz, Vr = VV[:, 0:out], VV[:, out:2 * out]
        nc.vector.tensor_copy(UVzr2[0:32, 0:32], Vz)
        nc.vector.tensor_copy(UVzr2[0:32, 64:96], Vr)
        nc.vector.tensor_copy(UVzr2[32:64, 32:64], Vz)
        nc.vector.tensor_copy(UVzr2[32:64, 96:128], Vr)
        # 8-row pieces: engine ops need 32-aligned partition bases, so stage
        # fp16 copies at base 0 and DMA them into the odd-base rows.
        Uz16 = cpool.tile([f_in, out], F16)
        Ur16 = cpool.tile([f_in, out], F16)
        Uh16 = cpool.tile([f_in, out], F16)
        nc.vector.tensor_copy(Uz16[:], Uz)
        nc.vector.tensor_copy(Ur16[:], Ur)
        nc.vector.tensor_copy(Uh16[:], Uh[:])
        nc.sync.dma_start(UVzr2[64:72, 0:32], Uz16[:])
        nc.sync.dma_start(UVzr2[64:72, 64:96], Ur16[:])
        nc.sync.dma_start(UVzr2[72:80, 32:64], Uz16[:])
        nc.sync.dma_start(UVzr2[72:80, 96:128], Ur16[:])
        nc.sync.dma_start(U2hs[64:72, 0:32], Uh16[:])
        nc.sync.dma_start(U2hs[72:80, 32:64], Uh16[:])
        nc.vector.tensor_copy(V2hs[0:32, 0:32], Vh32[:])
        nc.vector.tensor_copy(V2hs[32:64, 32:64], Vh32[:])
        nc.vector.tensor_copy(wp2s[0:32, 0:1], wsb["Wp"])
        nc.vector.tensor_copy(wp2s[32:64, 1:2], wsb["Wp"])
        nc.vector.tensor_copy(biasZ2[0:32, :], cbzr[0:32, :])
        nc.vector.tensor_copy(biasZ2[32:64, :], cbzr[0:32, :])
        nc.vector.tensor_copy(biasR2[0:32, :], cbzr[32:64, :])
        nc.vector.tensor_copy(biasR2[32:64, :], cbzr[32:64, :])
        nc.vector.tensor_copy(cbh2[0:32, :], cbh[:])
        nc.vector.tensor_copy(cbh2[32:64, :], cbh[:])
        nc.vector.tensor_copy(bp2[0:1, :], wsb["bp"])
        nc.sync.dma_start(bp2[1:2, :], wsb["bp"])
        # zero lhsT/rhs for the psum-clearing matmul of each block
        z96 = cpool.tile([1, feat], F16)
        z128 = cpool.tile([1, 128], F16)
        nc.vector.memset(z96[:], 0.0)
        nc.vector.memset(z128[:], 0.0)

        # ---------------- stage 1: deg -> dinvT [nblk, 128] ----------------
        dinvT = cpool.tile([nblk, 128], F32)
        with tc.tile_pool(name="degp", bufs=1) as dpool:
            wdg = dpool.tile([nblk, 128 * dmax], F32)
            nc.scalar.dma_start(wdg[:], wdegT[:])
            deg = dpool.tile([nblk, 128], F32)
            nc.vector.tensor_reduce(
                deg[:, :, None],
                wdg[:].rearrange("b (q d) -> b q d", d=dmax),
                axis=mybir.AxisListType.X, op=ALU.add)
            sq = dpool.tile([nblk, 128], F32)
            nc.scalar.activation(sq[:], deg[:], ACTF.Sqrt)
            nc.vector.reciprocal(dinvT[:], sq[:])
            # node-major pack: dinv_shard[b*128 + q] = dinvT[b, q]  (contiguous
            # per partition). NOTE: must be a GPSIMD (SWDGE) DMA — sync/HWDGE
            # DMAs writing a collective's input buffer deadlock in NRT.
            nc.gpsimd.dma_start(dinv_shard[:].rearrange("(b q) -> b q", q=128),
                                dinvT[:])

        # ---------------- stage 2: allgather dinv ---------------------------
        nc.gpsimd.collective_compute(
            "AllGather", ALU.bypass,
            ins=[dinv_shard[:]], outs=[dinv_all[:]],
            replica_groups=[list(range(ncores))])
        for c in range(ncores):
            nc.sync.dma_start(out=dinv_glob[c * npc:(c + 1) * npc],
                              in_=dinv_all[c * npcp:c * npcp + npc])
        if npad > n:  # zero the padding tail
            zt = cpool.tile([1, npad - n], F32)
            nc.vector.memset(zt[:], 0.0)
            nc.sync.dma_start(out=dinv_glob[n:npad, None], in_=zt[0:1, :])

        # ---------------- stage 2c: dinvd_all [96, npcp] fp16 ---------------
        # (local-only; overlaps the allgather) dinvd_all[f, n] = dinv[dst n]
        dinvd_all = cpool.tile([feat, npcp], F16)
        ones96f = cpool.tile([1, feat], F32)
        nc.gpsimd.memset(ones96f[:], 1.0)
        with tc.tile_pool(name="psdv", bufs=4, space="PSUM") as pdv, \
             tc.tile_pool(name="drow", bufs=2) as drp:
            for ci in range(npcp // 448):
                csl = slice(ci * 448, (ci + 1) * 448)
                drc = drp.tile([1, 448], F32, tag="drc")
                nc.sync.dma_start(drc[:], dinv_shard[None, csl])
                psd = pdv.tile([feat, 448], F32, tag="psd")
                nc.tensor.matmul(psd[:], lhsT=ones96f[:], rhs=drc[:],
                                 start=True, stop=True)
                nc.vector.tensor_copy(dinvd_all[:, csl], psd[:])

        # ---------------- stage 3: SpMM (gather + selection matmuls) --------
        # axt[t*8+f, node] = (A @ Y)[node, t*8+f] * dinv[dst]
        axt = cpool.tile([feat, npcp], F16)

        scb = cfg["scb"]
        lag = cfg["lag"]
        nsc = len(scb)
        assert sum(scb) == gblk
        scb0 = [sum(scb[:i]) for i in range(nsc + 1)]  # block offsets per group

        # block order: quarter q = g0-span + g1-span (GRU SC q depends on it)
        quarters = []
        for q in range(nsc):
            quarters.append(list(range(scb0[q], scb0[q + 1])) +
                            list(range(gblk + scb0[q], gblk + scb0[q + 1])))
        ord_blocks = [b for qs in quarters for b in qs]
        assert sorted(ord_blocks) == list(range(nblk))

        gpool = ctx.enter_context(tc.tile_pool(name="gat", bufs=3))
        bpool = ctx.enter_context(tc.tile_pool(name="bld", bufs=2))
        pgpool = ctx.enter_context(tc.tile_pool(name="ps_g", bufs=2, space="PSUM"))

        def emit_block(idxb, stb, j, b):
            Y = gpool.tile([128, eb * feat], F16, tag="Y")
            nc.gpsimd.indirect_dma_start(
                out=Y[:], out_offset=None,
                in_=x16[:, :],
                in_offset=bass.IndirectOffsetOnAxis(ap=idxb[:, j, :], axis=0))
            Yr = Y[:].rearrange("q (e f) -> q e f", f=feat)
            # per-edge dinv[src] (4 B gather) folded into the st tile
            dsr = gpool.tile([128, eb], F32, tag="dsr")
            nc.gpsimd.indirect_dma_start(
                out=dsr[:], out_offset=None,
                in_=dinv_glob[:, None],
                in_offset=bass.IndirectOffsetOnAxis(ap=idxb[:, j, :], axis=0))
            stj = stb[:, j, :].rearrange("q (e k) -> q e k", k=K)
            nc.vector.tensor_tensor(
                out=stj, in0=stj,
                in1=dsr[:, :, None].to_broadcast([128, eb, K]), op=ALU.mult)
            ps = pgpool.tile([feat, 128], F32, tag="psA")
            nc.tensor.matmul(ps[:], lhsT=z96[:], rhs=z128[:],
                             start=True, stop=False)
            for t in range(eb):
                w0 = int(r0[b, t])
                nc.tensor.matmul(ps[:, w0:w0 + K], lhsT=Yr[:, t, :],
                                 rhs=stb[:, j, t * K:(t + 1) * K],
                                 start=False, stop=(t == eb - 1))
            nc.vector.tensor_tensor(
                out=axt[:, b * 128:(b + 1) * 128], in0=ps[:],
                in1=dinvd_all[:, b * 128:(b + 1) * 128], op=ALU.mult)

        # one batched idx/st load per contiguous span (12-13 blocks)
        spmm_work = {}  # block -> (idxb_r, stb_r, j)
        for q in range(nsc):
            for g in range(2):
                b0 = g * gblk + scb0[q]
                nb = scb[q]
                nbmax = max(scb)
                idxb = bpool.tile([128, nbmax * eb], I32, tag="idxb")
                nc.scalar.dma_start(idxb[:, 0:nb * eb],
                                    gsrc[:, b0 * eb:(b0 + nb) * eb])
                stb = bpool.tile([128, nbmax * eb * K], F16, tag="stb")
                nc.scalar.dma_start(
                    stb[:, 0:nb * eb * K],
                    st16[:, b0 * eb * K:(b0 + nb) * eb * K])
                idxb_r = idxb[:].rearrange("q (b e) -> q b e", e=eb)
                stb_r = stb[:].rearrange("q (b e) -> q b e", e=eb * K)
                for j in range(nb):
                    spmm_work[b0 + j] = (idxb_r, stb_r, j)

        # ---------------- stage 4: GRU over time (wavefront of SCs) ---------
        grup = ctx.enter_context(tc.tile_pool(name="gru", bufs=1))
        pzrp = ctx.enter_context(tc.tile_pool(name="ps_zr", bufs=2, space="PSUM"))
        phpool = ctx.enter_context(tc.tile_pool(name="ps_h", bufs=2, space="PSUM"))

        XH2 = grup.tile([80, half], F16)    # rows 0:64 H (g0|g1), 64:80 ax
        XZ2 = grup.tile([2 * out, half], F16)
        RH2 = grup.tile([2 * out, half], F16)
        HT2 = grup.tile([2 * out, half], F16)
        acc2 = grup.tile([2 * out, half], F16)
        nc.vector.memset(XH2[:], 0.0)
        nc.vector.memset(acc2[:], 0.0)

        def sc_chunks(sc):
            w0 = scb0[sc] * 128
            w1 = scb0[sc + 1] * 128
            ch = []
            c = w0
            while c < w1:
                cw = min(gch, w1 - c)
                ch.append((c, cw))
                c += cw
            return w0, w1, ch

        def gru_step(sc, t):
            w0, w1, chunks = sc_chunks(sc)
            scs = slice(w0, w1)
            # ax rows: axg0 -> 64:72, axg1 -> 72:80
            eng = nc.sync
            eng.dma_start(XH2[64:72, scs],
                          axt[t * f_in:(t + 1) * f_in, w0:w1])
            eng.dma_start(XH2[72:80, scs],
                          axt[t * f_in:(t + 1) * f_in, half + w0:half + w1])
            for c0, cw in chunks:
                csl = slice(c0, c0 + cw)
                pzr = pzrp.tile([128, gch], F32, tag="pzr")
                nc.tensor.matmul(pzr[:, 0:cw], lhsT=UVzr2[:], rhs=XH2[:, csl],
                                 start=True, stop=True)
                nc.scalar.activation(XZ2[:, csl], pzr[0:64, 0:cw], ACTF.Sigmoid,
                                     bias=biasZ2[:, 0:1])
                nc.scalar.activation(RH2[:, csl], pzr[64:128, 0:cw], ACTF.Sigmoid,
                                     bias=biasR2[:, 0:1])
            # RH = R * H
            nc.vector.tensor_tensor(out=RH2[:, scs], in0=RH2[:, scs],
                                    in1=XH2[0:64, scs], op=ALU.mult)
            for c0, cw in chunks:
                csl = slice(c0, c0 + cw)
                ph = phpool.tile([2 * out, gch], F32, tag="ph")
                nc.tensor.matmul(ph[:, 0:cw], lhsT=U2hs[64:80, :],
                                 rhs=XH2[64:80, csl], start=True, stop=False)
                nc.tensor.matmul(ph[:, 0:cw], lhsT=V2hs[:], rhs=RH2[:, csl],
                                 start=False, stop=True)
                nc.scalar.activation(HT2[:, csl], ph[:, 0:cw], ACTF.Tanh,
                                     bias=cbh2[:, 0:1])
            # H' = Ht + Z*(H - Ht); acc += p_t * H'   (RH2 as scratch)
            nc.vector.tensor_tensor(out=RH2[:, scs], in0=XH2[0:64, scs],
                                    in1=HT2[:, scs], op=ALU.subtract)
            nc.vector.tensor_tensor(out=RH2[:, scs], in0=XZ2[:, scs],
                                    in1=RH2[:, scs], op=ALU.mult)
            nc.vector.tensor_tensor(out=XH2[0:64, scs], in0=HT2[:, scs],
                                    in1=RH2[:, scs], op=ALU.add)
            nc.vector.scalar_tensor_tensor(
                out=acc2[:, scs], in0=XH2[0:64, scs],
                scalar=pmat64[:, t:t + 1], in1=acc2[:, scs],
                op0=ALU.mult, op1=ALU.add)

        def gru_head(sc):
            w0, w1, chunks = sc_chunks(sc)
            scs = slice(w0, w1)
            nc.scalar.activation(HT2[:, scs], acc2[:, scs], ACTF.Relu)
            with tc.tile_pool(name=f"ps_d{sc}", bufs=1, space="PSUM") as pdpool, \
                 tc.tile_pool(name=f"ov{sc}", bufs=3) as ovpool:
                for c0, cw in chunks:
                    csl = slice(c0, c0 + cw)
                    pd = pdpool.tile([2, gch], F32, tag="pd")
                    nc.tensor.matmul(pd[:, 0:cw], lhsT=wp2s[:], rhs=HT2[:, csl],
                                     start=True, stop=True)
                    xcc = ovpool.tile([2, gch], F32, tag="xcc")
                    nc.sync.dma_start(xcc[:, 0:cw], xcol2[:, csl])
                    o2c = ovpool.tile([2, gch], F16, tag="o2c")
                    nc.vector.tensor_tensor(out=o2c[:, 0:cw], in0=pd[:, 0:cw],
                                            in1=xcc[:, 0:cw], op=ALU.add)
                    nc.vector.tensor_scalar(out=o2c[:, 0:cw], in0=o2c[:, 0:cw],
                                            scalar1=bp2[:, 0:1], scalar2=0.0,
                                            op0=ALU.add, op1=ALU.max)
                    nc.sync.dma_start(out_ext[:, csl], o2c[:, 0:cw])

        # wavefront: SC sc runs t-steps at slots sc*lag + t; quarter q+1's
        # SpMM blocks are emitted at the start of slot q*lag.
        emitted_q = 0
        for b in quarters[0]:
            emit_block(*spmm_work[b], b)
        emitted_q = 1
        n_slots = (nsc - 1) * lag + p
        for k in range(n_slots):
            if k % lag == 0 and emitted_q < nsc:
                for b in quarters[emitted_q]:
                    emit_block(*spmm_work[b], b)
                emitted_q += 1
            for sc in range(nsc):
                t = k - sc * lag
                if 0 <= t < p:
                    gru_step(sc, t)
                if t == p - 1:
                    gru_head(sc)

    return nc


TRACE = False
LAST_EXEC_TIME_NS = None
LAST_RESULT = None


def kernel(**inputs):
    global LAST_EXEC_TIME_NS, LAST_RESULT
    cfg = CFG
    x = np.asarray(inputs["x"], dtype=np.float32)
    in_maps, eb, dmax, npad, K, r0 = host_prep(x, inputs["edge_index"],
                                               inputs["edge_weight"], cfg)
    w = host_weights(inputs, cfg)
    for m in in_maps:
        m.update(w)
    nc = build_graph(cfg, eb, dmax, npad, K, r0)
    nc.finalize()

    from concourse.bass_utils import run_bass_kernel_spmd
    res = run_bass_kernel_spmd(nc, in_maps, core_ids=list(range(cfg["ncores"])),
                               trace=TRACE)
    LAST_EXEC_TIME_NS = res.exec_time_ns
    LAST_RESULT = res
    npc = cfg["npc"]
    outs = []
    for c in range(cfg["ncores"]):
        o = np.asarray(res.results[c]["out"], dtype=np.float32)  # [2, half]
        outs.append(o.reshape(-1)[:npc])
    return np.concatenate(outs).reshape(-1, 1).astype(np.float32)
